# revision 35
# baseline (speedup 1.0000x reference)
"""Trainium2 Bass kernel for nn_FCAutoEncoder (ragged_sequence).

Strategy:
  * Host: bucket rows by seq_length (5 sizes), split each bucket evenly
    across 8 cores (pure data parallel).  Per core, x is packed
    feature-major into ONE [128, sum_k t_k*c_k] bf16 tensor (t_k =
    SP[k]/128 K-tiles, c_k = columns of bucket k) so each bucket loads
    with a single DMA whose per-partition runs are t_k*c_k*2 bytes --
    the DMA engines are packet-rate-bound, so large contiguous runs
    matter more than raw bytes.  The output uses the same packed
    layout.
  * Host-side linear fusion: the per-size input scaler (expand) is
    folded into encoder layer 1 (F_k = We1 @ Win[k], restricted to the
    s_k live input features), and decoder layer 3 is folded into the
    per-size output scaler (B_k = Wout[k] @ Wd3, restricted to the s_k
    live output features).  This cuts tensor-engine work ~3x and weight
    DMA ~2.5x versus computing the 1008-wide expand/contract.
  * Device (per core, identical SPMD program): per bucket k:
      h1  = relu(F_k^T . x[:s_k] + fb_k)        [512]
      h2  = relu(We2 h1 + be2)                  [256]
      lat = We3 h2 + be3                        [128]
      d1  = relu(Wd1 lat + bd1)                 [256]
      d2  = relu(Wd2 d1 + bd2)                  [512]
      out = B_k d2 + bb_k                       [s_k]
    All matmul operands are bf16 (full PE rate, half DMA); PSUM
    accumulates fp32; bias(+ReLU) is fused into the PSUM evacuation on
    the Scalar/Vector engines.  The tail of bucket i (lat..out) is
    interleaved with the front of bucket i+1 so the PE never drains.
    Dummy warm-up matmuls run during the startup DMA window so the PE
    clock is ramped when real work arrives.  DMA triggers are spread
    across engines (Sync: x+bias, GpSimd: weights, Scalar: out stores)
    because trigger instructions serialize per engine at ~650ns each.
  * Host: unpack, scatter rows to original order.
"""
import os
import sys

sys.path.insert(0, "/opt/trn_rl_repo")

import numpy as np

SIZES = (36, 72, 144, 288, 1008)
SP = (128, 128, 256, 384, 1024)   # SIZES padded to multiples of 128
BASE = 1008
BASE_P = 1024
H1, H2, LAT = 512, 256, 128
N_CORES = 8
MAX_CHUNK = 512
ACT_BUFS = 40
WARM_MM = 3
# processing order: small bucket first (fast start), ascending so each
# bucket's inputs have time to stream in, small bucket last (short
# pipeline drain / small final store)
UNIT_ORDER = (0, 2, 3, 4, 1)

# matmul operand dtype: "bf16" (half DMA) or "f32r" (fp32 fallback)
W_DT = os.environ.get("KW_DT", "bf16")

_last_exec_ns = None
_prog_cache = {}


def _tiles(n, t=128):
    return [(s, min(t, n - s)) for s in range(0, n, t)]


def _chunks(c, maxn=MAX_CHUNK):
    """Split c (even) into even-sized chunks <= maxn."""
    if c <= 0:
        return []
    assert c % 2 == 0
    half = c // 2
    n = (c + maxn - 1) // maxn
    base, rem = divmod(half, n)
    out, off = [], 0
    for i in range(n):
        sz = 2 * (base + (1 if i < rem else 0))
        out.append((off, sz))
        off += sz
    return out


def _layout(c_ks):
    """Bucket processing order + packed x/out free-dim offsets."""
    order = [k for k in UNIT_ORDER if c_ks[k] > 0]
    offs = {}
    off = 0
    for k in order:
        offs[k] = off
        off += (SP[k] // 128) * c_ks[k]
    return order, offs, off


def _bias_layout():
    """Fixed column order of the packed [128, NB] bias tensor."""
    cols = []
    for k in range(5):
        for (js, jp) in _tiles(H1):
            cols.append(("front", k, js, jp))
    for (js, jp) in _tiles(H2):
        cols.append(("L2", 0, js, jp))
    for (js, jp) in _tiles(LAT):
        cols.append(("L3", 0, js, jp))
    for (js, jp) in _tiles(H2):
        cols.append(("D1", 0, js, jp))
    for (js, jp) in _tiles(H1):
        cols.append(("D2", 0, js, jp))
    for k in range(5):
        for (os_, op) in _tiles(SIZES[k]):
            cols.append(("out", k, os_, op))
    return cols


def _build_program(c_ks):
    import concourse.bacc as bacc
    import concourse.mybir as mybir
    from concourse import tile

    f32 = mybir.dt.float32
    f32r = mybir.dt.float32r
    bf16 = mybir.dt.bfloat16
    wdt = bf16 if W_DT == "bf16" else f32r    # matmul operand dtype
    AF = mybir.ActivationFunctionType
    ALU = mybir.AluOpType

    order, foffs, TOT = _layout(c_ks)

    bias_cols = _bias_layout()
    bias_idx = {c[:3]: i for i, c in enumerate(bias_cols)}

    def bcol(layer, k, start):
        return bias_idx[(layer, k, start)]

    nc = bacc.Bacc(None, target_bir_lowering=False, debug=False, num_devices=1)

    dram_xdt = bf16 if W_DT == "bf16" else f32
    xP = nc.dram_tensor("xP", [128, TOT], dram_xdt, kind="ExternalInput").ap()
    outP = nc.dram_tensor("outP", [128, TOT], dram_xdt,
                          kind="ExternalOutput").ap()
    dram_wdt = bf16 if W_DT == "bf16" else f32
    fTd = [
        nc.dram_tensor(f"fT{k}", [SP[k], H1], dram_wdt, kind="ExternalInput").ap()
        for k in range(5)
    ]
    bTd = [
        nc.dram_tensor(f"bT{k}", [H1, SIZES[k]], dram_wdt,
                       kind="ExternalInput").ap()
        for k in range(5)
    ]
    we2T = nc.dram_tensor("we2T", [H1, H2], dram_wdt, kind="ExternalInput").ap()
    we3T = nc.dram_tensor("we3T", [H2, LAT], dram_wdt, kind="ExternalInput").ap()
    wd1T = nc.dram_tensor("wd1T", [LAT, H2], dram_wdt, kind="ExternalInput").ap()
    wd2T = nc.dram_tensor("wd2T", [H2, H1], dram_wdt, kind="ExternalInput").ap()
    biasD = nc.dram_tensor("biases", [128, len(bias_cols)], f32,
                           kind="ExternalInput").ap()

    xPs = xP if W_DT == "bf16" else xP.bitcast(f32r)
    outPs = outP if W_DT == "bf16" else outP.bitcast(f32r)

    with tile.TileContext(nc) as tc:
        with (
            tc.tile_pool(name="wp", bufs=1) as wp,
            tc.tile_pool(name="ap", bufs=ACT_BUFS) as apool,
            tc.tile_pool(name="pp", bufs=8, space="PSUM") as pp,
        ):
            bias_t = wp.tile([128, len(bias_cols)], f32, tag="bias")

            def load_w(dram, n_rows, n_cols, tag, eng=None):
                """One batched DMA: [t*128, C] dram -> [128, t, C] tile.

                Weight triggers default to the otherwise-idle GpSimd
                engine; the first units' go on Sync (GpSimd's preamble
                drains would delay them ~2us).
                """
                t = n_rows // 128
                tl = wp.tile([128, t, n_cols], wdt, tag=tag)
                r = dram.rearrange("(t p) c -> p t c", p=128)
                if W_DT != "bf16":
                    r = r.bitcast(f32r)
                (eng or nc.gpsimd).dma_start(tl[:], r)
                return tl

            fT = {}
            bT = {}
            mid_t = {}
            xv = {}

            def load_xb(k, eng=None):
                """One DMA for a whole bucket's packed x block."""
                if k in xv:
                    return
                t = SP[k] // 128
                L = t * c_ks[k]
                tl = wp.tile([128, L], wdt, tag=f"x{k}")
                (eng or nc.sync).dma_start(tl[:], xPs[:, foffs[k]:foffs[k] + L])
                xv[k] = tl

            def xviews(k, c0, cn):
                c = c_ks[k]
                return [xv[k][:, i * c + c0:i * c + c0 + cn]
                        for i in range(SP[k] // 128)]

            def mid_load():
                if "we3" in mid_t:
                    return
                mid_t["we3"] = load_w(we3T, H2, LAT, "we3")
                mid_t["wd1"] = load_w(wd1T, LAT, H2, "wd1")
                mid_t["wd2"] = load_w(wd2T, H2, H1, "wd2")

            def evac(psum, mp, cn, bias_j, relu, eng, out_dt):
                o = apool.tile([mp, cn], out_dt, tag="act")
                b = bias_t[:mp, bias_j:bias_j + 1]
                if eng == "act":
                    nc.scalar.activation(
                        o[:], psum[:], AF.Relu if relu else AF.Identity, bias=b
                    )
                else:
                    if relu:
                        nc.vector.tensor_scalar(
                            o[:], psum[:], b, 0.0, ALU.add, ALU.max
                        )
                    else:
                        nc.vector.tensor_scalar_add(o[:], psum[:], b)
                return o

            def sub_layer(in_tiles, wtile, n_in, jtl, bias_layer, bias_k,
                          relu, eng, cn, out_dt=wdt):
                outs = []
                nkt = n_in // 128
                for (js, jp) in jtl:
                    psum = pp.tile([jp, cn], f32, tag="ps")
                    for i in range(nkt):
                        nc.tensor.matmul(
                            psum[:], wtile[:, i, js:js + jp], in_tiles[i][:],
                            start=(i == 0), stop=(i == nkt - 1),
                        )
                    e_i = ("dve" if (js // 128) % 2 == 0 else "act") \
                        if eng == "alt" else eng
                    outs.append(
                        evac(psum, jp, cn, bcol(bias_layer, bias_k, js),
                             relu, e_i, out_dt)
                    )
                return outs

            def emit_out(k, c0, cn, d2):
                tk = SP[k] // 128
                c = c_ks[k]
                ot = apool.tile([128, tk, cn], wdt, tag="outb", bufs=3)
                for ti, (os_, op) in enumerate(_tiles(SIZES[k])):
                    psum = pp.tile([op, cn], f32, tag="ps")
                    for i in range(H1 // 128):
                        nc.tensor.matmul(
                            psum[:], bT[k][:, i, os_:os_ + op], d2[i][:],
                            start=(i == 0), stop=(i == H1 // 128 - 1),
                        )
                    o = ot[0:op, ti, :]
                    b = bias_t[:op, bcol("out", k, os_):bcol("out", k, os_) + 1]
                    if (os_ // 128) % 2 == 0:
                        nc.vector.tensor_scalar_add(o, psum[:], b)
                    else:
                        nc.scalar.activation(o, psum[:], AF.Identity, bias=b)
                dst = outPs[:, foffs[k]:foffs[k] + tk * c] \
                    .rearrange("p (t c) -> p t c", c=c)[:, :, c0:c0 + cn]
                # last unit's store goes on Sync (idle by then) so it is
                # not queued behind earlier big stores on Scalar's queue
                eng = nc.sync if k == order[-1] else nc.scalar
                eng.dma_start(dst, ot[:])

            def tail_stages(k, c0, cn, h2):
                """Generator of tail stages; caller interleaves them."""
                lat = sub_layer(h2, mid_t["we3"], H2, _tiles(LAT), "L3", 0,
                                False, "dve", cn)
                yield
                d1 = sub_layer(lat, mid_t["wd1"], LAT, _tiles(H2), "D1", 0,
                               True, "alt", cn)
                yield
                d2 = sub_layer(d1, mid_t["wd2"], H2, _tiles(H1), "D2", 0,
                               True, "alt", cn)
                yield
                emit_out(k, c0, cn, d2)

            # units: (bucket, chunk_start, chunk_len) in processing order
            units = []
            for k in order:
                for (c0, cn) in _chunks(c_ks[k]):
                    units.append((k, c0, cn))

            jt1 = _tiles(H1)

            # ---- startup: first two units' inputs via Sync in need
            # order, then warm-up matmuls to ramp the PE clock while
            # the first real data lands
            # need-ordered startup loads.  Queues are packet-rate-bound
            # (~128 packets per transfer here), so x goes alone on
            # Sync's queue while the first weights + bias (which gate
            # every evacuation) go on Scalar's queue, which idles until
            # the out-stores ~25us in.
            su = [u[0] for u in units]
            for i, uk in enumerate(su[:2]):
                load_xb(uk)
                if uk not in fT:
                    fT[uk] = load_w(fTd[uk], SP[uk], H1, f"fT{uk}",
                                    eng=nc.scalar)
                if i == 0:
                    nc.scalar.dma_start(bias_t[:], biasD[:])
            mid_t["we2"] = load_w(we2T, H1, H2, "we2", eng=nc.scalar)
            if units:
                wml = wp.tile([128, 128], wdt, tag="warm_l")
                wmr = wp.tile([128, 512], wdt, tag="warm_r")
                nc.gpsimd.memset(wml[:], 0)
                nc.gpsimd.memset(wmr[:], 0)
                for _ in range(WARM_MM):
                    wps = pp.tile([128, 512], f32, tag="ps")
                    nc.tensor.matmul(wps[:], wml[:], wmr[:],
                                     start=True, stop=True)

            tail_prev = None
            h1pre = {}
            for ui, (k, c0, cn) in enumerate(units):
                first = ui == 0
                nxt = units[ui + 1] if ui + 1 < len(units) else None
                nxt2 = units[ui + 2] if ui + 2 < len(units) else None
                if k not in fT:
                    fT[k] = load_w(fTd[k], SP[k], H1, f"fT{k}")
                load_xb(k)
                xts = xviews(k, c0, cn)
                pre = h1pre.pop((k, c0), None)
                if pre is not None:
                    h1a, h1b = pre
                    if tail_prev is not None:
                        next(tail_prev, None)          # L3(prev)
                else:
                    # front: fused expand+encoder-L1, split for interleave
                    h1a = sub_layer(xts, fT[k], SP[k], jt1[:2], "front", k,
                                    True, "alt", cn)
                    if first:
                        mid_load()
                    if tail_prev is not None:
                        next(tail_prev, None)          # L3(prev)
                    h1b = sub_layer(xts, fT[k], SP[k], jt1[2:], "front", k,
                                    True, "alt", cn)
                if first and nxt is not None:
                    # fill the gap before L2_0 (nothing to interleave
                    # yet): emit the next unit's front matmuls now
                    nk, nc0, ncn = nxt
                    load_xb(nk)
                    nxts = xviews(nk, nc0, ncn)
                    nh1a = sub_layer(nxts, fT[nk], SP[nk], jt1[:2],
                                     "front", nk, True, "alt", ncn)
                    nh1b = sub_layer(nxts, fT[nk], SP[nk], jt1[2:],
                                     "front", nk, True, "alt", ncn)
                    h1pre[(nk, nc0)] = (nh1a, nh1b)
                # prefetch in need-order: next front weights + x, then
                # the one-after front weights, then this bucket's back
                if nxt is not None:
                    nk = nxt[0]
                    if nk not in fT:
                        fT[nk] = load_w(fTd[nk], SP[nk], H1, f"fT{nk}")
                    load_xb(nk)
                if tail_prev is not None:
                    next(tail_prev, None)              # D1(prev)
                if nxt2 is not None:
                    nk2 = nxt2[0]
                    if nk2 not in fT:
                        fT[nk2] = load_w(fTd[nk2], SP[nk2], H1, f"fT{nk2}")
                if k not in bT:
                    bT[k] = load_w(bTd[k], H1, SIZES[k], f"bT{k}")
                h2 = sub_layer(h1a + h1b, mid_t["we2"], H1, _tiles(H2),
                               "L2", 0, True, "alt", cn)
                if tail_prev is not None:
                    next(tail_prev, None)              # D2(prev)
                    next(tail_prev, None)              # out(prev)
                tail_prev = tail_stages(k, c0, cn, h2)

            if tail_prev is not None:
                for _ in tail_prev:
                    pass

    nc.compile()
    return nc


def _pad(a, shape):
    out = np.zeros(shape, dtype=np.float32)
    out[tuple(slice(0, s) for s in a.shape)] = a
    return out


def kernel(**inputs):
    global _last_exec_ns
    from concourse.bass_utils import run_bass_kernel_spmd
    import ml_dtypes

    wnp = ml_dtypes.bfloat16 if W_DT == "bf16" else np.float32

    x = np.asarray(inputs["x"], dtype=np.float32)
    seq = np.asarray(inputs["seq_lengths"]).astype(np.int64)
    B = x.shape[0]

    Win = np.asarray(inputs["Win"], dtype=np.float32)
    bin_ = np.asarray(inputs["bin_"], dtype=np.float32)
    Wout = np.asarray(inputs["Wout"], dtype=np.float32)
    bout = np.asarray(inputs["bout"], dtype=np.float32)
    We1 = np.asarray(inputs["We1"], dtype=np.float32)
    be1 = np.asarray(inputs["be1"], dtype=np.float32)
    We2 = np.asarray(inputs["We2"], dtype=np.float32)
    be2 = np.asarray(inputs["be2"], dtype=np.float32)
    We3 = np.asarray(inputs["We3"], dtype=np.float32)
    be3 = np.asarray(inputs["be3"], dtype=np.float32)
    Wd1 = np.asarray(inputs["Wd1"], dtype=np.float32)
    bd1 = np.asarray(inputs["bd1"], dtype=np.float32)
    Wd2 = np.asarray(inputs["Wd2"], dtype=np.float32)
    bd2 = np.asarray(inputs["bd2"], dtype=np.float32)
    Wd3 = np.asarray(inputs["Wd3"], dtype=np.float32)
    bd3 = np.asarray(inputs["bd3"], dtype=np.float32)

    # ---- bucket rows by size ----
    idx = [np.nonzero(seq == s)[0] for s in SIZES]
    n_ks = [len(i) for i in idx]
    c_ks = tuple(2 * (-(-n // (2 * N_CORES))) if n > 0 else 0 for n in n_ks)

    out = np.zeros((B, BASE), dtype=np.float32)
    if sum(c_ks) == 0:
        return out

    order, foffs, TOT = _layout(c_ks)

    # ---- host-fused shared weights (replicated across cores) ----
    shared = {}
    for k in range(5):
        s = SIZES[k]
        # front: h1 = relu(F_k x + fb_k), F_k = We1 @ Win[k][:, :s]
        fTk = (We1 @ Win[k][:, :s]).T          # [s, 512]
        shared[f"fT{k}"] = _pad(fTk, (SP[k], H1)).astype(wnp)
        # back: out = B_k d2 + bb_k, B_k = Wout[k][:s] @ Wd3
        bTk = (Wout[k][:s, :] @ Wd3).T          # [512, s]
        shared[f"bT{k}"] = np.ascontiguousarray(bTk).astype(wnp)
    shared["we2T"] = np.ascontiguousarray(We2.T).astype(wnp)
    shared["we3T"] = np.ascontiguousarray(We3.T).astype(wnp)
    shared["wd1T"] = np.ascontiguousarray(Wd1.T).astype(wnp)
    shared["wd2T"] = np.ascontiguousarray(Wd2.T).astype(wnp)

    bias_cols = _bias_layout()
    bp = np.zeros((128, len(bias_cols)), dtype=np.float32)
    fb = [We1 @ bin_[k] + be1 for k in range(5)]
    bb = [Wout[k][:SIZES[k], :] @ bd3 + bout[k][:SIZES[k]] for k in range(5)]
    vecs = {"L2": be2, "L3": be3, "D1": bd1, "D2": bd2}
    for j, col in enumerate(bias_cols):
        layer, k, start, width = col
        if layer == "front":
            v = fb[k][start:start + width]
        elif layer == "out":
            v = bb[k][start:start + width]
        else:
            v = vecs[layer][start:start + width]
        bp[: len(v), j] = v
    shared["biases"] = bp

    # ---- per-core packed x ----
    xdt = ml_dtypes.bfloat16 if W_DT == "bf16" else np.float32
    xc = x.astype(xdt) if xdt is not np.float32 else x
    in_maps = []
    core_rows = []
    for m in range(N_CORES):
        xPa = np.zeros((128, TOT), dtype=xdt)
        rows_info = []
        for k in order:
            c = c_ks[k]
            t = SP[k] // 128
            rows = idx[k][m * c:(m + 1) * c]
            A = np.zeros((c, t * 128), dtype=xdt)
            nf = min(SP[k], BASE)
            if len(rows):
                A[:len(rows), :nf] = xc[rows][:, :nf]
            xPa[:, foffs[k]:foffs[k] + t * c] = (
                A.reshape(c, t, 128).transpose(2, 1, 0).reshape(128, t * c)
            )
            rows_info.append((k, rows))
        in_maps.append({"xP": np.ascontiguousarray(xPa), **shared})
        core_rows.append(rows_info)

    # ---- build / fetch program ----
    key = (c_ks, W_DT)
    if key not in _prog_cache:
        _prog_cache[key] = _build_program(c_ks)
    nc = _prog_cache[key]

    trace = bool(os.environ.get("BASS_TRACE"))
    res = None
    last_exc = None
    for attempt in range(3):
        try:
            res = run_bass_kernel_spmd(
                nc, in_maps, list(range(N_CORES)), trace=trace
            )
            break
        except Exception as exc:  # rare NRT exec-unit flake / missing hook
            last_exc = exc
            trace = False
    if res is None:
        raise last_exc
    _last_exec_ns = res.exec_time_ns

    # ---- unpack / unsort (only the s_k live rows; rest stays zero) ----
    for m in range(N_CORES):
        oP = res.results[m]["outP"]
        for (k, rows) in core_rows[m]:
            if not len(rows):
                continue
            c = c_ks[k]
            t = SP[k] // 128
            s = SIZES[k]
            blk = oP[:, foffs[k]:foffs[k] + t * c] \
                .reshape(128, t, c).transpose(2, 1, 0).reshape(c, t * 128)
            out[rows, :s] = blk[:len(rows), :s].astype(np.float32)
    return out


# revision 36
# speedup vs baseline: 1.1939x; 1.1939x over previous
"""Trainium2 Bass kernel for nn_FCAutoEncoder (ragged_sequence).

Strategy:
  * Host: bucket rows by seq_length (5 sizes), split each bucket evenly
    across 8 cores (pure data parallel).  Per core, x is packed
    feature-major into ONE [128, sum_k t_k*c_k] bf16 tensor (t_k =
    SP[k]/128 K-tiles, c_k = columns of bucket k) so each bucket loads
    with a single DMA whose per-partition runs are t_k*c_k*2 bytes --
    the DMA engines are packet-rate-bound, so large contiguous runs
    matter more than raw bytes.  The output uses the same packed
    layout.
  * Host-side linear fusion: the per-size input scaler (expand) is
    folded into encoder layer 1 (F_k = We1 @ Win[k], restricted to the
    s_k live input features), and decoder layer 3 is folded into the
    per-size output scaler (B_k = Wout[k] @ Wd3, restricted to the s_k
    live output features).  This cuts tensor-engine work ~3x and weight
    DMA ~2.5x versus computing the 1008-wide expand/contract.
  * Device (per core, identical SPMD program): per bucket k:
      h1  = relu(F_k^T . x[:s_k] + fb_k)        [512]
      h2  = relu(We2 h1 + be2)                  [256]
      lat = We3 h2 + be3                        [128]
      d1  = relu(Wd1 lat + bd1)                 [256]
      d2  = relu(Wd2 d1 + bd2)                  [512]
      out = B_k d2 + bb_k                       [s_k]
    All matmul operands are bf16 (full PE rate, half DMA); PSUM
    accumulates fp32; bias(+ReLU) is fused into the PSUM evacuation on
    the Scalar/Vector engines.  The tail of bucket i (lat..out) is
    interleaved with the front of bucket i+1 so the PE never drains.
    Dummy warm-up matmuls run during the startup DMA window so the PE
    clock is ramped when real work arrives.  DMA triggers are spread
    across engines (Sync: x+bias, GpSimd: weights, Scalar: out stores)
    because trigger instructions serialize per engine at ~650ns each.
  * Host: unpack, scatter rows to original order.
"""
import os
import sys

sys.path.insert(0, "/opt/trn_rl_repo")

import numpy as np

SIZES = (36, 72, 144, 288, 1008)
SP = (128, 128, 256, 384, 1024)   # SIZES padded to multiples of 128
BASE = 1008
BASE_P = 1024
H1, H2, LAT = 512, 256, 128
N_CORES = 8
MAX_CHUNK = 512
ACT_BUFS = 40
WARM_MM = 3
# processing order: small bucket first (fast start), ascending so each
# bucket's inputs have time to stream in, small bucket last (short
# pipeline drain / small final store)
UNIT_ORDER = (0, 2, 3, 4, 1)

# matmul operand dtype: "bf16" (half DMA) or "f32r" (fp32 fallback)
W_DT = os.environ.get("KW_DT", "bf16")

_last_exec_ns = None
_prog_cache = {}


def _tiles(n, t=128):
    return [(s, min(t, n - s)) for s in range(0, n, t)]


def _chunks(c, maxn=MAX_CHUNK):
    """Split c (even) into even-sized chunks <= maxn."""
    if c <= 0:
        return []
    assert c % 2 == 0
    half = c // 2
    n = (c + maxn - 1) // maxn
    base, rem = divmod(half, n)
    out, off = [], 0
    for i in range(n):
        sz = 2 * (base + (1 if i < rem else 0))
        out.append((off, sz))
        off += sz
    return out


def _layout(c_ks):
    """Bucket processing order + packed x/out free-dim offsets."""
    order = [k for k in UNIT_ORDER if c_ks[k] > 0]
    offs = {}
    off = 0
    for k in order:
        offs[k] = off
        off += (SP[k] // 128) * c_ks[k]
    return order, offs, off


def _bias_layout():
    """Fixed column order of the packed [128, NB] bias tensor."""
    cols = []
    for k in range(5):
        for (js, jp) in _tiles(H1):
            cols.append(("front", k, js, jp))
    for (js, jp) in _tiles(H2):
        cols.append(("L2", 0, js, jp))
    for (js, jp) in _tiles(LAT):
        cols.append(("L3", 0, js, jp))
    for (js, jp) in _tiles(H2):
        cols.append(("D1", 0, js, jp))
    for (js, jp) in _tiles(H1):
        cols.append(("D2", 0, js, jp))
    for k in range(5):
        for (os_, op) in _tiles(SIZES[k]):
            cols.append(("out", k, os_, op))
    return cols


def _build_program(c_ks):
    import concourse.bacc as bacc
    import concourse.mybir as mybir
    from concourse import tile

    f32 = mybir.dt.float32
    f32r = mybir.dt.float32r
    bf16 = mybir.dt.bfloat16
    wdt = bf16 if W_DT == "bf16" else f32r    # matmul operand dtype
    AF = mybir.ActivationFunctionType
    ALU = mybir.AluOpType

    order, foffs, TOT = _layout(c_ks)

    bias_cols = _bias_layout()
    bias_idx = {c[:3]: i for i, c in enumerate(bias_cols)}

    def bcol(layer, k, start):
        return bias_idx[(layer, k, start)]

    nc = bacc.Bacc(None, target_bir_lowering=False, debug=False, num_devices=1)

    dram_xdt = bf16 if W_DT == "bf16" else f32
    xP = nc.dram_tensor("xP", [128, TOT], dram_xdt, kind="ExternalInput").ap()
    outP = nc.dram_tensor("outP", [128, TOT], dram_xdt,
                          kind="ExternalOutput").ap()
    dram_wdt = bf16 if W_DT == "bf16" else f32
    fTd = [
        nc.dram_tensor(f"fT{k}", [SP[k], H1], dram_wdt, kind="ExternalInput").ap()
        for k in range(5)
    ]
    bTd = [
        nc.dram_tensor(f"bT{k}", [H1, SIZES[k]], dram_wdt,
                       kind="ExternalInput").ap()
        for k in range(5)
    ]
    we2T = nc.dram_tensor("we2T", [H1, H2], dram_wdt, kind="ExternalInput").ap()
    we3T = nc.dram_tensor("we3T", [H2, LAT], dram_wdt, kind="ExternalInput").ap()
    wd1T = nc.dram_tensor("wd1T", [LAT, H2], dram_wdt, kind="ExternalInput").ap()
    wd2T = nc.dram_tensor("wd2T", [H2, H1], dram_wdt, kind="ExternalInput").ap()
    biasD = nc.dram_tensor("biases", [128, len(bias_cols)], f32,
                           kind="ExternalInput").ap()

    xPs = xP if W_DT == "bf16" else xP.bitcast(f32r)
    outPs = outP if W_DT == "bf16" else outP.bitcast(f32r)

    with tile.TileContext(nc) as tc:
        with (
            tc.tile_pool(name="wp", bufs=1) as wp,
            tc.tile_pool(name="ap", bufs=ACT_BUFS) as apool,
            tc.tile_pool(name="pp", bufs=8, space="PSUM") as pp,
        ):
            bias_t = wp.tile([128, len(bias_cols)], f32, tag="bias")

            def load_w(dram, n_rows, n_cols, tag, eng=None):
                """One batched DMA: [t*128, C] dram -> [128, t, C] tile.

                Weight triggers default to the otherwise-idle GpSimd
                engine; the first units' go on Sync (GpSimd's preamble
                drains would delay them ~2us).
                """
                t = n_rows // 128
                tl = wp.tile([128, t, n_cols], wdt, tag=tag)
                r = dram.rearrange("(t p) c -> p t c", p=128)
                if W_DT != "bf16":
                    r = r.bitcast(f32r)
                (eng or nc.gpsimd).dma_start(tl[:], r)
                return tl

            fT = {}
            bT = {}
            mid_t = {}
            xv = {}

            def load_xb(k, eng=None):
                """One DMA for a whole bucket's packed x block."""
                if k in xv:
                    return
                t = SP[k] // 128
                L = t * c_ks[k]
                tl = wp.tile([128, L], wdt, tag=f"x{k}")
                (eng or nc.sync).dma_start(tl[:], xPs[:, foffs[k]:foffs[k] + L])
                xv[k] = tl

            def xviews(k, c0, cn):
                c = c_ks[k]
                return [xv[k][:, i * c + c0:i * c + c0 + cn]
                        for i in range(SP[k] // 128)]

            def mid_load():
                if "we3" in mid_t:
                    return
                mid_t["we3"] = load_w(we3T, H2, LAT, "we3")
                mid_t["wd1"] = load_w(wd1T, LAT, H2, "wd1")
                mid_t["wd2"] = load_w(wd2T, H2, H1, "wd2")

            def evac(psum, mp, cn, bias_j, relu, eng, out_dt):
                o = apool.tile([mp, cn], out_dt, tag="act")
                b = bias_t[:mp, bias_j:bias_j + 1]
                if eng == "act":
                    nc.scalar.activation(
                        o[:], psum[:], AF.Relu if relu else AF.Identity, bias=b
                    )
                else:
                    if relu:
                        nc.vector.tensor_scalar(
                            o[:], psum[:], b, 0.0, ALU.add, ALU.max
                        )
                    else:
                        nc.vector.tensor_scalar_add(o[:], psum[:], b)
                return o

            def sub_layer(in_tiles, wtile, n_in, jtl, bias_layer, bias_k,
                          relu, eng, cn, out_dt=wdt):
                outs = []
                nkt = n_in // 128
                for (js, jp) in jtl:
                    psum = pp.tile([jp, cn], f32, tag="ps")
                    for i in range(nkt):
                        nc.tensor.matmul(
                            psum[:], wtile[:, i, js:js + jp], in_tiles[i][:],
                            start=(i == 0), stop=(i == nkt - 1),
                        )
                    e_i = ("dve" if (js // 128) % 2 == 0 else "act") \
                        if eng == "alt" else eng
                    outs.append(
                        evac(psum, jp, cn, bcol(bias_layer, bias_k, js),
                             relu, e_i, out_dt)
                    )
                return outs

            def emit_out(k, c0, cn, d2):
                tk = SP[k] // 128
                c = c_ks[k]
                ot = apool.tile([128, tk, cn], wdt, tag="outb", bufs=3)
                for ti, (os_, op) in enumerate(_tiles(SIZES[k])):
                    psum = pp.tile([op, cn], f32, tag="ps")
                    for i in range(H1 // 128):
                        nc.tensor.matmul(
                            psum[:], bT[k][:, i, os_:os_ + op], d2[i][:],
                            start=(i == 0), stop=(i == H1 // 128 - 1),
                        )
                    o = ot[0:op, ti, :]
                    b = bias_t[:op, bcol("out", k, os_):bcol("out", k, os_) + 1]
                    if (os_ // 128) % 2 == 0:
                        nc.vector.tensor_scalar_add(o, psum[:], b)
                    else:
                        nc.scalar.activation(o, psum[:], AF.Identity, bias=b)
                dst = outPs[:, foffs[k]:foffs[k] + tk * c] \
                    .rearrange("p (t c) -> p t c", c=c)[:, :, c0:c0 + cn]
                # last unit's store goes on Sync (idle by then) so it is
                # not queued behind earlier big stores on Scalar's queue
                eng = nc.sync if k == order[-1] else nc.scalar
                eng.dma_start(dst, ot[:])

            def tail_stages(k, c0, cn, h2):
                """Generator of tail stages; caller interleaves them."""
                lat = sub_layer(h2, mid_t["we3"], H2, _tiles(LAT), "L3", 0,
                                False, "dve", cn)
                yield
                d1 = sub_layer(lat, mid_t["wd1"], LAT, _tiles(H2), "D1", 0,
                               True, "alt", cn)
                yield
                d2 = sub_layer(d1, mid_t["wd2"], H2, _tiles(H1), "D2", 0,
                               True, "alt", cn)
                yield
                emit_out(k, c0, cn, d2)

            # units: (bucket, chunk_start, chunk_len) in processing order
            units = []
            for k in order:
                for (c0, cn) in _chunks(c_ks[k]):
                    units.append((k, c0, cn))

            jt1 = _tiles(H1)

            # ---- startup: first two units' inputs via Sync in need
            # order, then warm-up matmuls to ramp the PE clock while
            # the first real data lands
            # need-ordered startup loads on Sync; only the tiny bias
            # tensor (which gates every evacuation) goes on Scalar's
            # otherwise-idle queue so it lands in ~1us instead of
            # queueing behind ~600 packets of x/weight traffic.
            nc.scalar.dma_start(bias_t[:], biasD[:])
            su = [u[0] for u in units]
            for uk in su[:2]:
                load_xb(uk)
                if uk not in fT:
                    fT[uk] = load_w(fTd[uk], SP[uk], H1, f"fT{uk}",
                                    eng=nc.sync)
            mid_t["we2"] = load_w(we2T, H1, H2, "we2", eng=nc.sync)
            if units:
                wml = wp.tile([128, 128], wdt, tag="warm_l")
                wmr = wp.tile([128, 512], wdt, tag="warm_r")
                nc.gpsimd.memset(wml[:], 0)
                nc.gpsimd.memset(wmr[:], 0)
                for _ in range(WARM_MM):
                    wps = pp.tile([128, 512], f32, tag="ps")
                    nc.tensor.matmul(wps[:], wml[:], wmr[:],
                                     start=True, stop=True)

            tail_prev = None
            h1pre = {}
            for ui, (k, c0, cn) in enumerate(units):
                first = ui == 0
                nxt = units[ui + 1] if ui + 1 < len(units) else None
                nxt2 = units[ui + 2] if ui + 2 < len(units) else None
                if k not in fT:
                    fT[k] = load_w(fTd[k], SP[k], H1, f"fT{k}")
                load_xb(k)
                xts = xviews(k, c0, cn)
                pre = h1pre.pop((k, c0), None)
                if pre is not None:
                    h1a, h1b = pre
                    if tail_prev is not None:
                        next(tail_prev, None)          # L3(prev)
                else:
                    # front: fused expand+encoder-L1, split for interleave
                    h1a = sub_layer(xts, fT[k], SP[k], jt1[:2], "front", k,
                                    True, "alt", cn)
                    if first:
                        mid_load()
                    if tail_prev is not None:
                        next(tail_prev, None)          # L3(prev)
                    h1b = sub_layer(xts, fT[k], SP[k], jt1[2:], "front", k,
                                    True, "alt", cn)
                if first and nxt is not None:
                    # fill the gap before L2_0 (nothing to interleave
                    # yet): emit the next unit's front matmuls now
                    nk, nc0, ncn = nxt
                    load_xb(nk)
                    nxts = xviews(nk, nc0, ncn)
                    nh1a = sub_layer(nxts, fT[nk], SP[nk], jt1[:2],
                                     "front", nk, True, "alt", ncn)
                    nh1b = sub_layer(nxts, fT[nk], SP[nk], jt1[2:],
                                     "front", nk, True, "alt", ncn)
                    h1pre[(nk, nc0)] = (nh1a, nh1b)
                # prefetch in need-order: next front weights + x, then
                # the one-after front weights, then this bucket's back
                if nxt is not None:
                    nk = nxt[0]
                    if nk not in fT:
                        fT[nk] = load_w(fTd[nk], SP[nk], H1, f"fT{nk}")
                    load_xb(nk)
                if tail_prev is not None:
                    next(tail_prev, None)              # D1(prev)
                if nxt2 is not None:
                    nk2 = nxt2[0]
                    if nk2 not in fT:
                        fT[nk2] = load_w(fTd[nk2], SP[nk2], H1, f"fT{nk2}")
                if k not in bT:
                    bT[k] = load_w(bTd[k], H1, SIZES[k], f"bT{k}")
                h2 = sub_layer(h1a + h1b, mid_t["we2"], H1, _tiles(H2),
                               "L2", 0, True, "alt", cn)
                if tail_prev is not None:
                    next(tail_prev, None)              # D2(prev)
                    next(tail_prev, None)              # out(prev)
                tail_prev = tail_stages(k, c0, cn, h2)

            if tail_prev is not None:
                for _ in tail_prev:
                    pass

    nc.compile()
    return nc


def _pad(a, shape):
    out = np.zeros(shape, dtype=np.float32)
    out[tuple(slice(0, s) for s in a.shape)] = a
    return out


def kernel(**inputs):
    global _last_exec_ns
    from concourse.bass_utils import run_bass_kernel_spmd
    import ml_dtypes

    wnp = ml_dtypes.bfloat16 if W_DT == "bf16" else np.float32

    x = np.asarray(inputs["x"], dtype=np.float32)
    seq = np.asarray(inputs["seq_lengths"]).astype(np.int64)
    B = x.shape[0]

    Win = np.asarray(inputs["Win"], dtype=np.float32)
    bin_ = np.asarray(inputs["bin_"], dtype=np.float32)
    Wout = np.asarray(inputs["Wout"], dtype=np.float32)
    bout = np.asarray(inputs["bout"], dtype=np.float32)
    We1 = np.asarray(inputs["We1"], dtype=np.float32)
    be1 = np.asarray(inputs["be1"], dtype=np.float32)
    We2 = np.asarray(inputs["We2"], dtype=np.float32)
    be2 = np.asarray(inputs["be2"], dtype=np.float32)
    We3 = np.asarray(inputs["We3"], dtype=np.float32)
    be3 = np.asarray(inputs["be3"], dtype=np.float32)
    Wd1 = np.asarray(inputs["Wd1"], dtype=np.float32)
    bd1 = np.asarray(inputs["bd1"], dtype=np.float32)
    Wd2 = np.asarray(inputs["Wd2"], dtype=np.float32)
    bd2 = np.asarray(inputs["bd2"], dtype=np.float32)
    Wd3 = np.asarray(inputs["Wd3"], dtype=np.float32)
    bd3 = np.asarray(inputs["bd3"], dtype=np.float32)

    # ---- bucket rows by size ----
    idx = [np.nonzero(seq == s)[0] for s in SIZES]
    n_ks = [len(i) for i in idx]
    c_ks = tuple(2 * (-(-n // (2 * N_CORES))) if n > 0 else 0 for n in n_ks)

    out = np.zeros((B, BASE), dtype=np.float32)
    if sum(c_ks) == 0:
        return out

    order, foffs, TOT = _layout(c_ks)

    # ---- host-fused shared weights (replicated across cores) ----
    shared = {}
    for k in range(5):
        s = SIZES[k]
        # front: h1 = relu(F_k x + fb_k), F_k = We1 @ Win[k][:, :s]
        fTk = (We1 @ Win[k][:, :s]).T          # [s, 512]
        shared[f"fT{k}"] = _pad(fTk, (SP[k], H1)).astype(wnp)
        # back: out = B_k d2 + bb_k, B_k = Wout[k][:s] @ Wd3
        bTk = (Wout[k][:s, :] @ Wd3).T          # [512, s]
        shared[f"bT{k}"] = np.ascontiguousarray(bTk).astype(wnp)
    shared["we2T"] = np.ascontiguousarray(We2.T).astype(wnp)
    shared["we3T"] = np.ascontiguousarray(We3.T).astype(wnp)
    shared["wd1T"] = np.ascontiguousarray(Wd1.T).astype(wnp)
    shared["wd2T"] = np.ascontiguousarray(Wd2.T).astype(wnp)

    bias_cols = _bias_layout()
    bp = np.zeros((128, len(bias_cols)), dtype=np.float32)
    fb = [We1 @ bin_[k] + be1 for k in range(5)]
    bb = [Wout[k][:SIZES[k], :] @ bd3 + bout[k][:SIZES[k]] for k in range(5)]
    vecs = {"L2": be2, "L3": be3, "D1": bd1, "D2": bd2}
    for j, col in enumerate(bias_cols):
        layer, k, start, width = col
        if layer == "front":
            v = fb[k][start:start + width]
        elif layer == "out":
            v = bb[k][start:start + width]
        else:
            v = vecs[layer][start:start + width]
        bp[: len(v), j] = v
    shared["biases"] = bp

    # ---- per-core packed x ----
    xdt = ml_dtypes.bfloat16 if W_DT == "bf16" else np.float32
    xc = x.astype(xdt) if xdt is not np.float32 else x
    in_maps = []
    core_rows = []
    for m in range(N_CORES):
        xPa = np.zeros((128, TOT), dtype=xdt)
        rows_info = []
        for k in order:
            c = c_ks[k]
            t = SP[k] // 128
            rows = idx[k][m * c:(m + 1) * c]
            A = np.zeros((c, t * 128), dtype=xdt)
            nf = min(SP[k], BASE)
            if len(rows):
                A[:len(rows), :nf] = xc[rows][:, :nf]
            xPa[:, foffs[k]:foffs[k] + t * c] = (
                A.reshape(c, t, 128).transpose(2, 1, 0).reshape(128, t * c)
            )
            rows_info.append((k, rows))
        in_maps.append({"xP": np.ascontiguousarray(xPa), **shared})
        core_rows.append(rows_info)

    # ---- build / fetch program ----
    key = (c_ks, W_DT)
    if key not in _prog_cache:
        _prog_cache[key] = _build_program(c_ks)
    nc = _prog_cache[key]

    trace = bool(os.environ.get("BASS_TRACE"))
    res = None
    last_exc = None
    for attempt in range(3):
        try:
            res = run_bass_kernel_spmd(
                nc, in_maps, list(range(N_CORES)), trace=trace
            )
            break
        except Exception as exc:  # rare NRT exec-unit flake / missing hook
            last_exc = exc
            trace = False
    if res is None:
        raise last_exc
    _last_exec_ns = res.exec_time_ns

    # ---- unpack / unsort (only the s_k live rows; rest stays zero) ----
    for m in range(N_CORES):
        oP = res.results[m]["outP"]
        for (k, rows) in core_rows[m]:
            if not len(rows):
                continue
            c = c_ks[k]
            t = SP[k] // 128
            s = SIZES[k]
            blk = oP[:, foffs[k]:foffs[k] + t * c] \
                .reshape(128, t, c).transpose(2, 1, 0).reshape(c, t * 128)
            out[rows, :s] = blk[:len(rows), :s].astype(np.float32)
    return out
